# revision 7
# baseline (speedup 1.0000x reference)
"""CLIF spiking-neuron recurrence kernel for 8 Trainium2 NeuronCores.

Reference semantics (per element, T=64 sequential steps, gamma=0.5):
    u     = 0.5*u + x_t
    spike = (u >= 1.0)
    m     = s_prev * sigmoid(0.5*u) + spike
    s     = sigmoid(m)                       # carried (in-place sigmoid_)
    u     = u - spike*(1.0 + s)
Output: spikes [T, B, D] float32.

v3 design:
- Pure data parallel: 65536 elements/core as [128 x 512], G=2 column
  groups of 256 for pipelining; V = 2^t * u in ONE [128,512] PSUM bank
  (power-of-2 scaling exact in fp32; leak folds into ACT's free scale).
- The reset avoids sigmoid(m): 1+sigmoid(1+q), q = s_prev*sg, is a
  quadratic with the constant term CONSTRAINED to the bf16-representable
  b0'=1.734375 (fit on q in [0.30,0.93] - for t>=1 spiking elements
  always have q >= 0.311 since s_prev>=0.5, sg>=c). t=0 has q == 0
  exactly and uses a one-off fp32 path with the exact constant.
  Offline fp32 simulation of this exact arithmetic (incl. fp16/bf16
  rounding): 454/33.5M spike flips, rel err 1.02e-2 < 2e-2. Prior sims
  predicted HW flip counts exactly (1/1, 18/18).
- Critical cycle is 3 hops: ACT(sg) -> DVE(Y3) -> PE(V += W_t @ y).
  CLIF_Y3 emits y = (sg>=c)*((C2*q+C1)*q+1) in fp16; the reset matmul
  is bf16 stationary (-2^t*b0' exact diagonals) x fp16 moving = 1-pass,
  exact products in fp32 PSUM accum.
- Input adds are bf16 hi+lo split matmuls (exact to 2^-17 of x), issued
  TWO steps ahead so the PE's weight self-loads never block the cycle.
- M = s_prev*sg + spike (wide DVE) and s = sigmoid(M) (wide ACT) run off
  the cycle with a full step of slack.
- Output: M as bf16 via the SWDGE cast DMA path (spike=1 <=> M>=1 with
  margin 0.55 vs 1.0, preserved under any rounding). Host compares >= 1.
- DMA batched in 4-step chunks; inputs ride the SP HWDGE ring, weights
  the ACT ring, output casts on SWDGE.
"""

import sys
import types

import numpy as np
import ml_dtypes

# If BASS_TRACE is set but the image's antenv lacks axon_hooks,
# run_bass_kernel_spmd would crash importing it; install a null-hook
# module so tracing degrades gracefully instead.
try:
    import antenv.axon_hooks  # noqa: F401
except Exception:
    try:
        import antenv
        _hooks = types.ModuleType("antenv.axon_hooks")
        _hook_cell = [None]
        _hooks.set_axon_ntff_profile_hook = (
            lambda h: _hook_cell.__setitem__(0, h))
        _hooks.get_axon_ntff_profile_hook = lambda: _hook_cell[0]
        sys.modules["antenv.axon_hooks"] = _hooks
        antenv.axon_hooks = _hooks
    except Exception:
        pass

import concourse.bass as bass
import concourse.bacc as bacc
import concourse.mybir as mybir
import concourse.tile as tile
import concourse.dve_ops as dve_ops
from concourse.dve_spec import Spec, Src0, Src1, C0, C1, C2, One, lower, _has_src1
from concourse.dve_uop import DveOpSpec
from concourse.bass_utils import run_bass_kernel_spmd

F32 = mybir.dt.float32
BF16 = mybir.dt.bfloat16
FP16 = mybir.dt.float16
AF = mybir.ActivationFunctionType

T = 64
B = 128
D = 4096
N_CORES = 8
P = 128
NPC = B * D // N_CORES          # 65536 elements per core
FDT = NPC // P                  # 512 free columns per core
CHUNK = 4                       # steps per input/output DMA chunk
NCHUNK = T // CHUNK

GROUPS = [(0, 256), (256, 256)]

# Constrained LS fit of f(q) = 1 + sigmoid(1 + q) on q in [0.30, 0.93]
# with constant term forced to the bf16-exact B0P (max err 6.9e-4; only
# reachable q values matter - see module docstring).
B0P = 1.734375                          # bf16-exact
B1C = 0.18530899
B2C = -0.03826911
YC1 = float(np.float32(B1C / B0P))
YC2 = float(np.float32(B2C / B0P))
B0_EXACT = float(np.float32(1.0 + 1.0 / (1.0 + np.exp(-1.0))))

_NC_CACHE = None
LAST_RESULTS = None


def _register_dve_op(name, spec):
    for op in dve_ops.OPS:
        if op.name == name:
            return op
    shas = {}
    for ver in ("v3", "v4"):
        u = lower(spec, ver=ver)
        shas[ver] = DveOpSpec(name=name, opcode=1, uops=u,
                              rd1_en=_has_src1(spec)).sha(ver)
    op = dve_ops.DveOp(name, spec, subdim=False, uops_sha=shas)
    dve_ops.OPS.append(op)
    dve_ops._SUB_OPCODE_FOR_NAME[name] = (
        dve_ops._CUSTOM_DVE_ROW_BASE + len(dve_ops.OPS) - 1)
    dve_ops.CUSTOM_DVE_SPECS[name] = spec
    return op


# M = s_prev*sg + (sg >= c)          in0=s_prev, in1=sg, s0=c
CLIF_M = _register_dve_op("CLIF_M_ANT", Spec(
    body=Src0 * Src1 + (Src1 >= C0),
    reference=lambda in0, in1, s0, s1, imm2:
        in0 * in1 + (in1 >= s0).astype(np.float32),
))

# y = (sg >= c) * ((C2*q + C1)*q + 1),  q = s_prev*sg
#     in0=s_prev, in1=sg, s0=c, s1=C1, imm2=C2
_q = Src0 * Src1
CLIF_Y3 = _register_dve_op("CLIF_Y3_ANT", Spec(
    body=(Src1 >= C0) * ((C2 * _q + C1) * _q + One),
    reference=lambda in0, in1, s0, s1, imm2:
        (in1 >= s0).astype(np.float32)
        * ((imm2 * (in0 * in1) + s1) * (in0 * in1) + 1.0),
))


def _build():
    nc = bacc.Bacc(None, target_bir_lowering=False, debug=False,
                   num_devices=N_CORES)

    xsh = nc.declare_dram_parameter("xsh", [NCHUNK, P, CHUNK, FDT], BF16,
                                    isOutput=False)
    xsl = nc.declare_dram_parameter("xsl", [NCHUNK, P, CHUNK, FDT], BF16,
                                    isOutput=False)
    wt = nc.declare_dram_parameter("wt", [P, P], BF16, isOutput=False)   # identity
    w0 = nc.declare_dram_parameter("w0", [P, P], F32, isOutput=False)    # -B0*I
    wts = nc.declare_dram_parameter("wts", [P, T, P], BF16,
                                    isOutput=False)  # -2^t*B0P diagonals
    out = nc.declare_dram_parameter("out", [NCHUNK, P, CHUNK, FDT], BF16,
                                    isOutput=True)

    with tile.TileContext(nc) as tc:
        with (
            tc.tile_pool(name="wpool", bufs=1) as wpool,
            tc.tile_pool(name="cpool", bufs=1) as cpool,
            tc.tile_pool(name="xpool", bufs=2) as xpool,
            tc.tile_pool(name="mpool", bufs=2) as mpool,
            tc.tile_pool(name="sgpool", bufs=3) as sgpool,
            tc.tile_pool(name="spool", bufs=3) as spool,
            tc.tile_pool(name="ypool", bufs=4) as ypool,
            tc.tile_pool(name="vpool", bufs=1, space="PSUM") as vpool,
        ):
            # --- one-time setup -------------------------------------------
            eyeb = wpool.tile([P, P], BF16, tag="eye")
            nc.sync.dma_start(eyeb[:], wt[:])
            w0t = wpool.tile([P, P], F32, tag="w0")
            nc.scalar.dma_start(w0t[:], w0[:])
            wtile = wpool.tile([P, T, P], BF16, tag="wts")
            nc.scalar.dma_start(wtile[:, 0:8, :], wts[:, 0:8, :])
            nc.scalar.dma_start(wtile[:, 8:T, :], wts[:, 8:T, :])

            halft = cpool.tile([P, 1], F32, tag="half")
            nc.gpsimd.memset(halft[:], 0.5)
            ct = cpool.tile([P, 1], F32, tag="c")
            # c = sigmoid_LUT(0.5), same LUT as the per-step sigmoids
            nc.scalar.activation(ct[:], halft[:], AF.Sigmoid, bias=0.0, scale=1.0)
            c_ap = ct[:, 0:1]

            # --- initial state --------------------------------------------
            sw_prev = spool.tile([P, FDT], F32, tag="sw")
            nc.gpsimd.memset(sw_prev[:], 0.0)

            V = vpool.tile([P, FDT], F32, tag="V")

            # PE warm-up: dummy matmuls so the HAM clock gate reaches
            # 2.4 GHz before the first real matmul
            junk = vpool.tile([P, 128], F32, tag="junk")
            for _ in range(10):
                nc.tensor.matmul(junk[:], eyeb[:], eyeb[:], start=True, stop=True)

            xtiles = {}
            xh0 = xpool.tile([P, CHUNK, FDT], BF16, tag="xh")
            xl0 = xpool.tile([P, CHUNK, FDT], BF16, tag="xl")
            nc.sync.dma_start(xh0[:], xsh[0])
            nc.sync.dma_start(xl0[:], xsl[0])
            xtiles[0] = (xh0, xl0)

            # input for step 0 (in-loop adds run one step ahead)
            nc.tensor.matmul(V[:], eyeb[:], xh0[:, 0, :],
                             start=True, stop=False, skip_group_check=True)
            nc.tensor.matmul(V[:], eyeb[:], xl0[:, 0, :],
                             start=False, stop=False, skip_group_check=True)

            # --- the recurrence -------------------------------------------
            mb = None
            for t in range(T):
                ci = t % CHUNK
                k = t // CHUNK
                if ci == 0:
                    mb = mpool.tile([P, CHUNK, FDT], F32, tag="m")
                    if k + 1 < NCHUNK:
                        xh = xpool.tile([P, CHUNK, FDT], BF16, tag="xh")
                        xl = xpool.tile([P, CHUNK, FDT], BF16, tag="xl")
                        nc.sync.dma_start(xh[:], xsh[k + 1])
                        nc.sync.dma_start(xl[:], xsl[k + 1])
                        xtiles[k + 1] = (xh, xl)
                        xtiles.pop(k - 1, None)

                sc_sg = float(2.0 ** (-t - 1))

                # critical cycle: sg -> y3 -> reset matmul
                sgw = sgpool.tile([P, FDT], F32, tag="sg")
                for o, w in GROUPS:
                    nc.scalar.activation(sgw[:, o:o + w], V[:, o:o + w],
                                         AF.Sigmoid, bias=0.0, scale=sc_sg)

                # input adds for step t+1, PER GROUP: emitted right after the
                # sg reads of V (Tile orders the WAR edges correctly); they
                # run on the PE while the DVE does Y3, off the cycle
                if t + 1 < T:
                    t1 = t + 1
                    xh, xl = xtiles[t1 // CHUNK]
                    for o, w in GROUPS:
                        nc.tensor.matmul(V[:, o:o + w], eyeb[:],
                                         xh[:, t1 % CHUNK, o:o + w],
                                         start=False, stop=False,
                                         skip_group_check=True)
                        nc.tensor.matmul(V[:, o:o + w], eyeb[:],
                                         xl[:, t1 % CHUNK, o:o + w],
                                         start=False, stop=False,
                                         skip_group_check=True)

                if t < T - 1:
                    ydt = F32 if t == 0 else FP16
                    ys = []
                    for o, w in GROUPS:
                        y = ypool.tile([P, w], ydt, tag=f"y{o}")
                        nc.vector._custom_dve(CLIF_Y3, out=y[:],
                                              in0=sw_prev[:, o:o + w],
                                              in1=sgw[:, o:o + w],
                                              s0=c_ap, s1=YC1, imm2=YC2)
                        ys.append(y)
                    wsrc = w0t[:] if t == 0 else wtile[:, t, :]
                    for gi, (o, w) in enumerate(GROUPS):
                        nc.tensor.matmul(V[:, o:o + w], wsrc, ys[gi][:],
                                         start=False,
                                         stop=(t == T - 2 and gi == len(GROUPS) - 1),
                                         skip_group_check=True)

                # off-cycle, PER GROUP (keeps the two chains staggered so
                # engine queues stay busy and semaphore latency hides):
                # M (the output; also feeds s) and s = sigmoid(M)
                for o, w in GROUPS:
                    nc.vector._custom_dve(CLIF_M, out=mb[:, ci, o:o + w],
                                          in0=sw_prev[:, o:o + w],
                                          in1=sgw[:, o:o + w], s0=c_ap)
                if t < T - 1:
                    sw_new = spool.tile([P, FDT], F32, tag="sw")
                    for o, w in GROUPS:
                        nc.scalar.activation(sw_new[:, o:o + w],
                                             mb[:, ci, o:o + w], AF.Sigmoid,
                                             bias=0.0, scale=1.0)
                    sw_prev = sw_new

                if ci == CHUNK - 1:
                    # bf16 cast-on-DMA (SWDGE): half the output bytes
                    nc.gpsimd.dma_start(out[k], mb[:])

    nc.compile()
    return nc


def _get_nc():
    global _NC_CACHE
    if _NC_CACHE is None:
        _NC_CACHE = _build()
    return _NC_CACHE


def kernel(x_seq: np.ndarray) -> np.ndarray:
    global LAST_RESULTS
    x = np.ascontiguousarray(x_seq, dtype=np.float32)
    assert x.shape == (T, B, D), x.shape

    # 2^t prescale (exact in fp32), bf16 hi/lo split, per-core shard,
    # chunk-major layout
    scale = 2.0 ** np.arange(T, dtype=np.float64)
    xsc = (x.reshape(T, -1).astype(np.float64) * scale[:, None]).astype(np.float32)
    xh = xsc.astype(ml_dtypes.bfloat16)
    xl = (xsc - xh.astype(np.float32)).astype(ml_dtypes.bfloat16)
    xh = xh.reshape(T, N_CORES, P, FDT)
    xl = xl.reshape(T, N_CORES, P, FDT)

    eye_host = np.eye(P, dtype=np.float32).astype(ml_dtypes.bfloat16)
    w0_host = (-B0_EXACT * np.eye(P, dtype=np.float32)).astype(np.float32)
    w_host = np.zeros((P, T, P), dtype=np.float32)
    diag_vals = (-(2.0 ** np.arange(T, dtype=np.float64)) * B0P).astype(np.float32)
    pi = np.arange(P)
    w_host[pi[:, None], np.arange(T)[None, :], pi[:, None]] = diag_vals[None, :]
    w_host = w_host.astype(ml_dtypes.bfloat16)

    nc = _get_nc()
    in_maps = []
    for c in range(N_CORES):
        xhc = xh[:, c].reshape(NCHUNK, CHUNK, P, FDT).transpose(0, 2, 1, 3)
        xlc = xl[:, c].reshape(NCHUNK, CHUNK, P, FDT).transpose(0, 2, 1, 3)
        in_maps.append({
            "xsh": np.ascontiguousarray(xhc),
            "xsl": np.ascontiguousarray(xlc),
            "wt": eye_host,
            "w0": w0_host,
            "wts": w_host,
        })
    LAST_RESULTS = run_bass_kernel_spmd(nc, in_maps, list(range(N_CORES)))

    full = np.empty((T, N_CORES, P, FDT), dtype=np.float32)
    for c in range(N_CORES):
        res = LAST_RESULTS.results[c]
        m = np.asarray(res["out"]).astype(np.float32)       # [NCHUNK,P,CHUNK,FDT]
        m = m.transpose(0, 2, 1, 3).reshape(T, P, FDT)
        full[:, c] = (m >= 1.0).astype(np.float32)
    return full.reshape(T, B, D)


# revision 8
# speedup vs baseline: 1.1827x; 1.1827x over previous
"""CLIF spiking-neuron recurrence kernel for 8 Trainium2 NeuronCores.

Reference semantics (per element, T=64 sequential steps, gamma=0.5):
    u     = 0.5*u + x_t
    spike = (u >= 1.0)
    m     = s_prev * sigmoid(0.5*u) + spike
    s     = sigmoid(m)                       # carried (in-place sigmoid_)
    u     = u - spike*(1.0 + s)
Output: spikes [T, B, D] float32.

Strategy:
- Pure data parallel over the B*D = 524288 elements: 65536 per core as
  [128 partitions x 512 free], split into G independent pipeline groups
  along the free dim. Each group's step is a serial dependency loop
  (sigmoid -> CLIF_M -> sigmoid -> CLIF_Y -> matmul); with the input
  matmul hoisted off that loop, the kernel is latency-bound at
  T * loop-latency, and groups overlap on the engines.
- The membrane potential lives in PSUM as V_t = 2^t * u_t (power-of-2
  scaling is exact in fp32; 2^63*|u|max is far below fp32 range). The
  leak folds into per-step constants; the input add V += I @ (2^t x_t)
  runs on the TensorEngine right after step t-1's sigmoid read, off the
  critical loop; the reset matmul V += I @ y closes the loop.
- Two custom DVE ops (registered into the per-NEFF uop table, verified
  bit-exact on HW) fuse all elementwise work into 2 Vector ops:
    CLIF_M: M = s_prev*sg + (sg >= c)            (q-mult + spike + add)
    CLIF_Y: y = (sg >= c) * (s*(-2^t) + (-2^t))  [= -2^t*spike*(1+s)]
  where sg = sigmoid(2^-(t+1) * V) and c = sigmoidLUT(0.5) is computed
  on-device once; the ACT LUT is strictly monotone around z=0.5
  (verified on HW), so (sg >= c) <=> (u >= 1) exactly.
- One wide [128,512] input DMA and one wide output DMA per step.
- The kernel streams out sg (fp32); the host applies spike = (sg >= c),
  bit-identical to the on-device compares.
"""

import sys
import types

import numpy as np
import ml_dtypes

# If BASS_TRACE is set but the image's antenv lacks axon_hooks,
# run_bass_kernel_spmd would crash importing it; install a null-hook
# module so tracing degrades gracefully instead.
try:
    import antenv.axon_hooks  # noqa: F401
except Exception:
    try:
        import antenv
        _hooks = types.ModuleType("antenv.axon_hooks")
        _hook_cell = [None]
        _hooks.set_axon_ntff_profile_hook = (
            lambda h: _hook_cell.__setitem__(0, h))
        _hooks.get_axon_ntff_profile_hook = lambda: _hook_cell[0]
        sys.modules["antenv.axon_hooks"] = _hooks
        antenv.axon_hooks = _hooks
    except Exception:
        pass

import concourse.bass as bass
import concourse.bacc as bacc
import concourse.mybir as mybir
import concourse.tile as tile
import concourse.dve_ops as dve_ops
from concourse.dve_spec import Spec, Src0, Src1, C0, C1, lower, _has_src1
from concourse.dve_uop import DveOpSpec
from concourse.bass_utils import run_bass_kernel_spmd

F32 = mybir.dt.float32
AF = mybir.ActivationFunctionType
ALU = mybir.AluOpType

T = 64
B = 128
D = 4096
N_CORES = 8
P = 128
NPC = B * D // N_CORES          # 65536 elements per core
FDT = NPC // P                  # 512 free columns per core

# group column ranges (start, width) along the 512-wide free dim
GROUPS = [(0, 256), (256, 256)]

_NC_CACHE = None
LAST_RESULTS = None


def _register_dve_op(name, spec):
    for op in dve_ops.OPS:
        if op.name == name:
            return op
    shas = {}
    for ver in ("v3", "v4"):
        u = lower(spec, ver=ver)
        shas[ver] = DveOpSpec(name=name, opcode=1, uops=u,
                              rd1_en=_has_src1(spec)).sha(ver)
    op = dve_ops.DveOp(name, spec, subdim=False, uops_sha=shas)
    dve_ops.OPS.append(op)
    dve_ops._SUB_OPCODE_FOR_NAME[name] = (
        dve_ops._CUSTOM_DVE_ROW_BASE + len(dve_ops.OPS) - 1)
    dve_ops.CUSTOM_DVE_SPECS[name] = spec
    return op


# M = s_prev*sg + (sg >= c)          in0=s_prev, in1=sg, s0=c
CLIF_M = _register_dve_op("CLIF_M_ANT", Spec(
    body=Src0 * Src1 + (Src1 >= C0),
    reference=lambda in0, in1, s0, s1, imm2:
        in0 * in1 + (in1 >= s0).astype(np.float32),
))
# y = (sg >= c) * (s*zneg + zneg)    in0=s, in1=sg, s0=c, s1=zneg=-2^t
CLIF_Y = _register_dve_op("CLIF_Y_ANT", Spec(
    body=(Src1 >= C0) * (Src0 * C1 + C1),
    reference=lambda in0, in1, s0, s1, imm2:
        (in1 >= s0).astype(np.float32) * (in0 * s1 + s1),
))


def _build():
    nc = bacc.Bacc(None, target_bir_lowering=False, debug=False,
                   num_devices=N_CORES)

    xs = nc.declare_dram_parameter("xs", [T, P, FDT], F32, isOutput=False)
    wt = nc.declare_dram_parameter("wt", [P, P], F32, isOutput=False)  # identity
    out = nc.declare_dram_parameter("out", [T, P, FDT], F32, isOutput=True)
    cout = nc.declare_dram_parameter("cout", [P, 1], F32, isOutput=True)

    G = len(GROUPS)
    with tile.TileContext(nc) as tc:
        with (
            tc.tile_pool(name="wpool", bufs=1) as wpool,
            tc.tile_pool(name="cpool", bufs=1) as cpool,
            tc.tile_pool(name="xpool", bufs=8) as xpool,
            tc.tile_pool(name="sgpool", bufs=6) as sgpool,
            tc.tile_pool(name="spool", bufs=4) as spool,
            tc.tile_pool(name="mpool", bufs=4) as mpool,
            tc.tile_pool(name="ypool", bufs=4) as ypool,
            tc.tile_pool(name="vpool", bufs=1, space="PSUM") as vpool,
        ):
            # --- one-time setup -------------------------------------------
            eye = wpool.tile([P, P], F32, tag="eye")
            nc.sync.dma_start(eye[:], wt[:])

            halft = cpool.tile([P, 1], F32, tag="half")
            nc.gpsimd.memset(halft[:], 0.5)
            ct = cpool.tile([P, 1], F32, tag="c")
            # c = sigmoid_LUT(0.5), same LUT as the per-step sigmoids
            nc.scalar.activation(ct[:], halft[:], AF.Sigmoid, bias=0.0, scale=1.0)
            nc.sync.dma_start(cout[:], ct[:])
            c_ap = ct[:, 0:1]

            # --- initial state --------------------------------------------
            V = []
            s_prev = []
            for g, (o, w) in enumerate(GROUPS):
                s0 = spool.tile([P, w], F32, tag=f"s{g}")
                nc.gpsimd.memset(s0[:], 0.0)
                s_prev.append(s0)
                vt = vpool.tile([P, w], F32, tag=f"V{g}")
                V.append(vt)

            # PE warm-up: dummy matmuls fill the otherwise-idle prologue
            # window so the HAM clock gate reaches 2.4 GHz before the first
            # real matmul (the first ~8 steps otherwise run at 1.2 GHz)
            junk = vpool.tile([P, 128], F32, tag="junk")
            for _ in range(10):
                nc.tensor.matmul(junk[:], eye[:], eye[:], start=True, stop=True)

            x0 = xpool.tile([P, FDT], F32, tag="x")
            nc.sync.dma_start(x0[:], xs[0])
            for g, (o, w) in enumerate(GROUPS):
                nc.tensor.matmul(V[g][:], eye[:], x0[:, o:o + w],
                                 start=True, stop=False, skip_group_check=True)

            # --- the recurrence -------------------------------------------
            for t in range(T):
                sc_sg = float(2.0 ** (-t - 1))
                zneg = float(-(2.0 ** t))

                # one wide input prefetch per step (all groups)
                if t < T - 1:
                    xnext = xpool.tile([P, FDT], F32, tag="x")
                    nc.sync.dma_start(xnext[:], xs[t + 1])

                # one wide output tile per step; ACT fills per-group slices.
                # Ops are emitted stage-grouped across groups so no engine's
                # FIFO head-of-line blocks an independent group's work.
                sgw = sgpool.tile([P, FDT], F32, tag="sg")
                for g, (o, w) in enumerate(GROUPS):
                    nc.scalar.activation(sgw[:, o:o + w], V[g][:], AF.Sigmoid,
                                         bias=0.0, scale=sc_sg)

                if t < T - 1:
                    # input add for the NEXT step: off the critical loop,
                    # legal as soon as this step's sigmoid has read V
                    for g, (o, w) in enumerate(GROUPS):
                        nc.tensor.matmul(V[g][:], eye[:], xnext[:, o:o + w],
                                         start=False, stop=False,
                                         skip_group_check=True)

                    # M = s_prev*sg + spike
                    msbs = []
                    for g, (o, w) in enumerate(GROUPS):
                        msb = mpool.tile([P, w], F32, tag=f"m{g}")
                        nc.vector._custom_dve(CLIF_M, out=msb[:],
                                              in0=s_prev[g][:],
                                              in1=sgw[:, o:o + w], s0=c_ap)
                        msbs.append(msb)

                    # s = sigmoid(M)
                    for g, (o, w) in enumerate(GROUPS):
                        s_new = spool.tile([P, w], F32, tag=f"s{g}")
                        nc.scalar.activation(s_new[:], msbs[g][:], AF.Sigmoid,
                                             bias=0.0, scale=1.0)
                        s_prev[g] = s_new

                    # y = -2^t * spike * (1+s) ; V += I @ y closes the loop
                    for g, (o, w) in enumerate(GROUPS):
                        y = ypool.tile([P, w], F32, tag=f"y{g}")
                        nc.vector._custom_dve(CLIF_Y, out=y[:],
                                              in0=s_prev[g][:],
                                              in1=sgw[:, o:o + w],
                                              s0=c_ap, s1=zneg)
                        nc.tensor.matmul(V[g][:], eye[:], y[:],
                                         start=False, stop=(t + 1 == T - 1),
                                         skip_group_check=True)

                nc.sync.dma_start(out[t], sgw[:])

    nc.compile()
    return nc


def _get_nc():
    global _NC_CACHE
    if _NC_CACHE is None:
        _NC_CACHE = _build()
    return _NC_CACHE


def kernel(x_seq: np.ndarray) -> np.ndarray:
    global LAST_RESULTS
    x = np.ascontiguousarray(x_seq, dtype=np.float32)
    assert x.shape == (T, B, D), x.shape

    # 2^t prescale (exact in fp32) and per-core shard [T, P, FDT]
    scale = (2.0 ** np.arange(T, dtype=np.float64)).astype(np.float32)
    xsc = x.reshape(T, -1) * scale[:, None]
    xsc = xsc.reshape(T, N_CORES, P, FDT)

    eye_host = np.eye(P, dtype=np.float32)

    nc = _get_nc()
    in_maps = [
        {"xs": np.ascontiguousarray(xsc[:, c]), "wt": eye_host}
        for c in range(N_CORES)
    ]
    LAST_RESULTS = run_bass_kernel_spmd(nc, in_maps, list(range(N_CORES)))

    full = np.empty((T, N_CORES, P, FDT), dtype=np.float32)
    for c in range(N_CORES):
        res = LAST_RESULTS.results[c]
        c_val = np.asarray(res["cout"], dtype=np.float32)[0, 0]
        sg = np.asarray(res["out"], dtype=np.float32)
        full[:, c] = (sg >= c_val).astype(np.float32)
    return full.reshape(T, B, D)



# revision 13
# speedup vs baseline: 1.3338x; 1.1278x over previous
"""CLIF spiking-neuron recurrence kernel for 8 Trainium2 NeuronCores.

Reference semantics (per element, T=64 sequential steps, gamma=0.5):
    u     = 0.5*u + x_t
    spike = (u >= 1.0)
    m     = s_prev * sigmoid(0.5*u) + spike
    s     = sigmoid(m)                       # carried (in-place sigmoid_)
    u     = u - spike*(1.0 + s)
Output: spikes [T, B, D] float32.

v5 design (baseline G=3 saturation regime + cheap primitives):
- Pure data parallel: 65536 elements/core as [128 x 512], G=3 column
  groups; V_g = 2^t * u in per-group PSUM tiles (power-of-2 scaling is
  exact in fp32; the leak folds into the ACT's free scale).
- Critical cycle per group is 3 hops: ACT(sg_g) -> DVE(Y3_g) ->
  PE(V_g += W_t @ y_g).  CLIF_Y3 computes the reset magnitude from
  q = s_prev*sg directly:  y = (sg>=c) * ((C2*q + C1)*q + 1)  where
  (C2*q+C1)*q+1 ~ (1+sigmoid(1+q))/b0' (constrained LS fit, err 6.9e-4
  only over the reachable q >= 0.311 for t>=1).  The constant's scale
  -2^t*b0' rides in bf16-exact diagonal matmul weights; y is fp16, so
  the reset matmul is a cheap 1-pass bf16xfp16 with exact products in
  fp32 PSUM accumulation.  t=0 (where q == 0 exactly) uses a one-off
  fp32 path with the exact constant -(1+sigmoid(1)).
- M = s_prev*sg + spike and s = sigmoid(M) run OFF the cycle as single
  wide [128,512] ops (one DVE + one ACT instruction per step instead of
  three), with a full step of slack; they couple the groups only with a
  one-step lag, so the three per-group cycles still stagger and keep
  the engine queues saturated (which is what hides the ~0.6us
  cross-engine semaphore latency - measured in earlier traces).
- Input adds stay exact per-group fp32 matmuls, emitted right after the
  sg reads so they run while the DVE does Y3.
- Output: M as bf16 via the SWDGE cast DMA (spike=1 <=> M>=1 with
  margin 0.55 vs 1.0 under any rounding).  Host compares >= 1.
- DMA batched in 4-step/1MB chunks (input on the SP HWDGE ring, weights
  on the ACT ring, output casts on SWDGE/gpsimd).
- Offline fp32 simulation of this exact arithmetic (incl. fp16
  rounding): 445/33.5M spike flips, rel err 1.01e-2 < 2e-2.  The same
  simulator predicted HW flip counts exactly on three prior variants.
"""

import sys
import types

import numpy as np
import ml_dtypes

# If BASS_TRACE is set but the image's antenv lacks axon_hooks,
# run_bass_kernel_spmd would crash importing it; install a null-hook
# module so tracing degrades gracefully instead.
try:
    import antenv.axon_hooks  # noqa: F401
except Exception:
    try:
        import antenv
        _hooks = types.ModuleType("antenv.axon_hooks")
        _hook_cell = [None]
        _hooks.set_axon_ntff_profile_hook = (
            lambda h: _hook_cell.__setitem__(0, h))
        _hooks.get_axon_ntff_profile_hook = lambda: _hook_cell[0]
        sys.modules["antenv.axon_hooks"] = _hooks
        antenv.axon_hooks = _hooks
    except Exception:
        pass

import concourse.bass as bass
import concourse.bacc as bacc
import concourse.mybir as mybir
import concourse.tile as tile
import concourse.dve_ops as dve_ops
from concourse.dve_spec import Spec, Src0, Src1, C0, C1, C2, One, lower, _has_src1
from concourse.dve_uop import DveOpSpec
from concourse.bass_utils import run_bass_kernel_spmd

F32 = mybir.dt.float32
BF16 = mybir.dt.bfloat16
FP16 = mybir.dt.float16
AF = mybir.ActivationFunctionType

T = 64
B = 128
D = 4096
N_CORES = 8
P = 128
NPC = B * D // N_CORES          # 65536 elements per core
FDT = NPC // P                  # 512 free columns per core
CHUNK = 4                       # steps per input/output DMA chunk
NCHUNK = T // CHUNK

GROUPS = [(0, 172), (172, 172), (344, 168)]

# Constrained LS fit of f(q) = 1 + sigmoid(1 + q) on q in [0.30, 0.93]
# with constant term forced to the bf16-exact B0P.
B0P = 1.734375                          # bf16-exact
B1C = 0.18530899
B2C = -0.03826911
YC1 = float(np.float32(B1C / B0P))
YC2 = float(np.float32(B2C / B0P))
B0_EXACT = float(np.float32(1.0 + 1.0 / (1.0 + np.exp(-1.0))))

_NC_CACHE = None
LAST_RESULTS = None


def _register_dve_op(name, spec):
    for op in dve_ops.OPS:
        if op.name == name:
            return op
    shas = {}
    for ver in ("v3", "v4"):
        u = lower(spec, ver=ver)
        shas[ver] = DveOpSpec(name=name, opcode=1, uops=u,
                              rd1_en=_has_src1(spec)).sha(ver)
    op = dve_ops.DveOp(name, spec, subdim=False, uops_sha=shas)
    dve_ops.OPS.append(op)
    dve_ops._SUB_OPCODE_FOR_NAME[name] = (
        dve_ops._CUSTOM_DVE_ROW_BASE + len(dve_ops.OPS) - 1)
    dve_ops.CUSTOM_DVE_SPECS[name] = spec
    return op


# M = s_prev*sg + (sg >= c)          in0=s_prev, in1=sg, s0=c
CLIF_M = _register_dve_op("CLIF_M_ANT", Spec(
    body=Src0 * Src1 + (Src1 >= C0),
    reference=lambda in0, in1, s0, s1, imm2:
        in0 * in1 + (in1 >= s0).astype(np.float32),
))

# y = (sg >= c) * ((C2*q + C1)*q + 1),  q = s_prev*sg
#     in0=s_prev, in1=sg, s0=c, s1=C1, imm2=C2
_q = Src0 * Src1
CLIF_Y3 = _register_dve_op("CLIF_Y3_ANT", Spec(
    body=(Src1 >= C0) * ((C2 * _q + C1) * _q + One),
    reference=lambda in0, in1, s0, s1, imm2:
        (in1 >= s0).astype(np.float32)
        * ((imm2 * (in0 * in1) + s1) * (in0 * in1) + 1.0),
))


def _build():
    nc = bacc.Bacc(None, target_bir_lowering=False, debug=False,
                   num_devices=N_CORES)

    xs = nc.declare_dram_parameter("xs", [NCHUNK, P, CHUNK, FDT], F32,
                                   isOutput=False)
    wt = nc.declare_dram_parameter("wt", [P, P], F32, isOutput=False)   # identity
    w0 = nc.declare_dram_parameter("w0", [P, P], F32, isOutput=False)   # -B0*I
    wts = nc.declare_dram_parameter("wts", [P, T, P], BF16,
                                    isOutput=False)  # -2^t*B0P diagonals
    out = nc.declare_dram_parameter("out", [NCHUNK, P, CHUNK, FDT], BF16,
                                    isOutput=True)

    with tile.TileContext(nc) as tc:
        with (
            tc.tile_pool(name="wpool", bufs=1) as wpool,
            tc.tile_pool(name="cpool", bufs=1) as cpool,
            tc.tile_pool(name="xpool", bufs=2) as xpool,
            tc.tile_pool(name="mpool", bufs=2) as mpool,
            tc.tile_pool(name="sgpool", bufs=4) as sgpool,
            tc.tile_pool(name="spool", bufs=4) as spool,
            tc.tile_pool(name="ypool", bufs=6) as ypool,
            tc.tile_pool(name="vpool", bufs=1, space="PSUM") as vpool,
        ):
            # --- one-time setup -------------------------------------------
            eye = wpool.tile([P, P], F32, tag="eye")
            nc.sync.dma_start(eye[:], wt[:])
            w0t = wpool.tile([P, P], F32, tag="w0")
            nc.scalar.dma_start(w0t[:], w0[:])
            wtile = wpool.tile([P, T, P], BF16, tag="wts")
            nc.scalar.dma_start(wtile[:, 0:8, :], wts[:, 0:8, :])
            nc.scalar.dma_start(wtile[:, 8:T, :], wts[:, 8:T, :])

            halft = cpool.tile([P, 1], F32, tag="half")
            nc.gpsimd.memset(halft[:], 0.5)
            ct = cpool.tile([P, 1], F32, tag="c")
            # c = sigmoid_LUT(0.5), same LUT as the per-step sigmoids
            nc.scalar.activation(ct[:], halft[:], AF.Sigmoid, bias=0.0, scale=1.0)
            c_ap = ct[:, 0:1]

            # --- initial state --------------------------------------------
            sw_prev = spool.tile([P, FDT], F32, tag="sw")
            nc.gpsimd.memset(sw_prev[:], 0.0)

            V = []
            for g, (o, w) in enumerate(GROUPS):
                vt = vpool.tile([P, w], F32, tag=f"V{g}")
                V.append(vt)

            # PE warm-up: dummy matmuls so the HAM clock gate ramps before
            # the first real matmul
            junk = vpool.tile([P, 128], F32, tag="junk")
            for _ in range(10):
                nc.tensor.matmul(junk[:], eye[:], eye[:], start=True, stop=True)

            xtiles = {}
            x0 = xpool.tile([P, CHUNK, FDT], F32, tag="x")
            nc.sync.dma_start(x0[:], xs[0])
            xtiles[0] = x0

            for g, (o, w) in enumerate(GROUPS):
                nc.tensor.matmul(V[g][:], eye[:], x0[:, 0, o:o + w],
                                 start=True, stop=False, skip_group_check=True)

            # --- the recurrence -------------------------------------------
            mb = None
            for t in range(T):
                ci = t % CHUNK
                k = t // CHUNK
                if ci == 0:
                    mb = mpool.tile([P, CHUNK, FDT], F32, tag="m")
                    if k + 1 < NCHUNK:
                        xn = xpool.tile([P, CHUNK, FDT], F32, tag="x")
                        nc.sync.dma_start(xn[:], xs[k + 1])
                        xtiles[k + 1] = xn
                        xtiles.pop(k - 1, None)

                sc_sg = float(2.0 ** (-t - 1))

                # critical cycle: sg_g -> y3_g -> reset matmul_g
                sgw = sgpool.tile([P, FDT], F32, tag="sg")
                for g, (o, w) in enumerate(GROUPS):
                    nc.scalar.activation(sgw[:, o:o + w], V[g][:],
                                         AF.Sigmoid, bias=0.0, scale=sc_sg)

                # input adds for step t+1: off the critical loop, run on the
                # PE while the DVE does Y3
                if t + 1 < T:
                    t1 = t + 1
                    xn = xtiles[t1 // CHUNK]
                    for g, (o, w) in enumerate(GROUPS):
                        nc.tensor.matmul(V[g][:], eye[:],
                                         xn[:, t1 % CHUNK, o:o + w],
                                         start=False, stop=False,
                                         skip_group_check=True)

                if t < T - 1:
                    ydt = F32 if t == 0 else FP16
                    ys = []
                    for g, (o, w) in enumerate(GROUPS):
                        y = ypool.tile([P, w], ydt, tag=f"y{g}")
                        nc.vector._custom_dve(CLIF_Y3, out=y[:],
                                              in0=sw_prev[:, o:o + w],
                                              in1=sgw[:, o:o + w],
                                              s0=c_ap, s1=YC1, imm2=YC2)
                        ys.append(y)
                    wsrc = w0t[:] if t == 0 else wtile[:, t, :]
                    for g, (o, w) in enumerate(GROUPS):
                        nc.tensor.matmul(V[g][:], wsrc, ys[g][:],
                                         start=False,
                                         stop=(t == T - 2 and g == len(GROUPS) - 1),
                                         skip_group_check=True)

                # off-cycle, WIDE (1 instr instead of 3; couples the groups
                # only with a one-step lag): M and s = sigmoid(M)
                nc.vector._custom_dve(CLIF_M, out=mb[:, ci, :],
                                      in0=sw_prev[:], in1=sgw[:], s0=c_ap)
                if t < T - 1:
                    sw_new = spool.tile([P, FDT], F32, tag="sw")
                    nc.scalar.activation(sw_new[:], mb[:, ci, :], AF.Sigmoid,
                                         bias=0.0, scale=1.0)
                    sw_prev = sw_new

                if ci == CHUNK - 1:
                    # bf16 cast-on-DMA (SWDGE): half the output bytes
                    nc.gpsimd.dma_start(out[k], mb[:])

    nc.compile()
    return nc


def _get_nc():
    global _NC_CACHE
    if _NC_CACHE is None:
        _NC_CACHE = _build()
    return _NC_CACHE


def kernel(x_seq: np.ndarray) -> np.ndarray:
    global LAST_RESULTS
    x = np.ascontiguousarray(x_seq, dtype=np.float32)
    assert x.shape == (T, B, D), x.shape

    # 2^t prescale (exact in fp32), per-core shard, chunk-major layout
    scale = (2.0 ** np.arange(T, dtype=np.float64)).astype(np.float32)
    xsc = x.reshape(T, -1) * scale[:, None]
    xsc = xsc.reshape(T, N_CORES, P, FDT)

    eye_host = np.eye(P, dtype=np.float32)
    w0_host = (-B0_EXACT * np.eye(P, dtype=np.float32)).astype(np.float32)
    w_host = np.zeros((P, T, P), dtype=np.float32)
    diag_vals = (-(2.0 ** np.arange(T, dtype=np.float64)) * B0P).astype(np.float32)
    pi = np.arange(P)
    w_host[pi[:, None], np.arange(T)[None, :], pi[:, None]] = diag_vals[None, :]
    w_host = w_host.astype(ml_dtypes.bfloat16)

    nc = _get_nc()
    in_maps = []
    for c in range(N_CORES):
        xc = xsc[:, c].reshape(NCHUNK, CHUNK, P, FDT).transpose(0, 2, 1, 3)
        in_maps.append({
            "xs": np.ascontiguousarray(xc),
            "wt": eye_host,
            "w0": w0_host,
            "wts": w_host,
        })
    LAST_RESULTS = run_bass_kernel_spmd(nc, in_maps, list(range(N_CORES)))

    full = np.empty((T, N_CORES, P, FDT), dtype=np.float32)
    for c in range(N_CORES):
        res = LAST_RESULTS.results[c]
        m = np.asarray(res["out"]).astype(np.float32)       # [NCHUNK,P,CHUNK,FDT]
        m = m.transpose(0, 2, 1, 3).reshape(T, P, FDT)
        full[:, c] = (m >= 1.0).astype(np.float32)
    return full.reshape(T, B, D)
